# revision 54
# baseline (speedup 1.0000x reference)
"""Expert-parallel grouped-GEMM FFN (MoE expert module) for TRN2, 8 NeuronCores.

Problem: xs [16384, 1024] grouped contiguously into 16 experts x 1024 tokens.
Per expert e: y = relu(x @ w1[e].T + b1[e]) @ w2[e].T + b2[e].

Sharding: expert-parallel, 2 experts per core. Each core computes its two
experts' FFN independently; outputs are disjoint row-blocks of the result, so
no collectives are needed.

Precision: weights and activations are bf16 (host-side cast), accumulation and
biases fp32.  l2 relative error ~3e-3, well inside the 2e-2 gate.

The matmul stream runs at the N=512 issue floor (~212 ns/MM at the measured
2.417 GHz), i.e. the 2048-MM stream is already at the hardware cadence.
Measured overhead decomposition (exec ~461.3us vs 433.8us stream floor):
~7.2us framework preamble (fixed cross-engine barrier; engine queues cannot
issue DMAs before it ends), ~6.3us startup DMA ramp, ~9.0us of 41 periodic
432ns PE stalls with an exact 10.79us period (external timer interference -
unchanged by instruction/DMA/notification counts; unfixable from the
kernel), and ~4.8us tail (eviction + writeback + fixed teardown drain).

Findings baked into this version (measured on HW, see session notes):
  - Per-queue DMA rate is row-size-bound (~2KB rows 175 GB/s, 4KB ~205,
    8KB ~390): bulk transfers use >=8KB dram rows - w1 in 4-k-tile groups,
    w2 in 2-dd groups, y per (expert, token-half), cutting DMA packets ~3x.
  - Only the sync ring's DGE starts promptly; scalar/gpsimd rings lag
    ~3us on a cold first fetch, so ALL startup-critical transfers ride the
    sync ring serially in deadline order (w1[0], x-nt0 as one 8KB-row
    transfer, w1[1..3], the k4..7 group, x-nt1; the head k-tiles are
    contiguous dram duplicates in `w1h`).  single_packet=True batches the
    descriptor stream (2KB-row head runs ~250 GB/s instead of ~175).
  - Tile dependencies are tile-granular (a reader waits for ALL writers of
    a tile), so each startup-critical transfer gets its own tile.
  - The warmup (NX-paced ~107ns each, cold) covers exactly the
    preamble->first-data window (~12.8us); under-covering by >3.4us causes
    a HAM MID-window re-throttle to 1.2 GHz and ~2us of cold real chains.
  - fp8 cannot pass the 2e-2 gate: e4m3 end-to-end measures 5.0e-2 l2
    (mm2-only 3.5e-2), and DoubleRow is only ~1.44x - mixed/residual
    schemes need >=3 fp8 MMs per replaced bf16 MM, a net loss.
  - Tail: the last d-tile runs nt0 N=512, then nt1 as N=384 + N=128 chains
    in separate PSUM banks so the post-final-matmul drainage (evict + DMA +
    receipt) covers only 128 columns.

Per-core schedule (per expert, all matmuls N=512, 128-contraction):
  - mm1: for each of 32 h-tiles k, two 8-matmul PSUM chains (contraction D),
    ACT evicts relu(acc + b1) -> h[k] bf16 in SBUF.  The second token-half
    chain trails DEFER k-tiles so expert 0's first chains need only the first
    token-half of x.
  - mm2: for each of 8 d-tiles, two interleaved 32-matmul PSUM chains
    (contraction H); ACT evicts acc + b2 into a per-token-half y tile that is
    written back in one DMA per (expert, token-half).
  - Startup-critical transfers ride one sync-ring in priority order; w2
    groups are issued mid-mm1 so the 2MB transfers hide under the trailing
    mm1 chains.
"""

import numpy as np
import ml_dtypes

import concourse.bacc as bacc
import concourse.mybir as mybir
import concourse.tile as tile
from concourse.bass_utils import run_bass_kernel_spmd

P = 128                 # SBUF partitions / PE array dim
D = 1024                # model dim
H = 4096                # hidden dim
E = 16                  # experts
N_TOK = 16384           # total tokens
N_CORES = 8
E_LOC = E // N_CORES    # experts per core = 2
NE = N_TOK // E         # tokens per expert = 1024
DC = D // P             # 8  (d chunks: mm1 contraction / mm2 output)
HC = H // P             # 32 (h chunks)
NT = 512                # matmul moving free dim (one PSUM bank of fp32)
NN = NE // NT           # 2  (token tiles per expert)
DEFER = 6               # k-tiles the second token-half chain trails by
W1G = 4                 # k-tiles per batched w1 DMA (8KB dram rows)
NG1 = HC // W1G         # 8 w1 groups
HEAD = 4                # expert 0's first k-tiles ride as single-k DMAs
W2G = 2                 # dd-tiles per batched w2 DMA (16KB dram rows)
NG2 = DC // W2G         # 4 w2 groups
WARM_MM = 50            # HAM warmup matmuls (cover preamble->first-DMA-landing)

F32 = mybir.dt.float32
BF16 = mybir.dt.bfloat16
NP_BF16 = ml_dtypes.bfloat16

_CACHE = {}


def _build_nc():
    nc = bacc.Bacc(None, target_bir_lowering=False)

    # Host-tiled layouts (see _prep_in_maps for the exact index maps):
    #   xt  [s, nt, p, c, j]      = x_e[nt*512+j, c*128+p]          (8KB rows)
    #   w1t [s, g, p, u, c, j]    = w1[e, (g*4+u)*128+j, c*128+p]   (8KB rows)
    #   w2t [s, g, p, v, k, j]    = w2[e, (g*2+v)*128+j, k*128+p]   (16KB rows)
    #   br  [s, p, 0:HC]          = b1[e, k*128+p]
    #   br  [s, p, HC+dd]         = b2[e, dd*128+p]
    #   yt  [s, nt, p, dd, j]     = y_e[nt*512+j, dd*128+p]         (8KB rows)
    xt = nc.dram_tensor("xt", [E_LOC, NN, P, DC, NT], BF16, kind="ExternalInput")
    # Contiguous duplicates of expert 0's first HEAD w1 k-tiles: the startup-
    # critical stream rides the sync ring as per-k single-tile transfers
    # (every other ring's DGE lags ~3us on its cold first fetch).
    w1h = nc.dram_tensor("w1h", [HEAD, P, DC, P], BF16, kind="ExternalInput")
    w1t = nc.dram_tensor("w1t", [E_LOC, NG1, P, W1G, DC, P], BF16,
                         kind="ExternalInput")
    w2t = nc.dram_tensor("w2t", [E_LOC, NG2, P, W2G, HC, P], BF16,
                         kind="ExternalInput")
    br = nc.dram_tensor("br", [E_LOC, P, HC + DC], F32, kind="ExternalInput")
    yt = nc.dram_tensor("yt", [E_LOC, NN, P, DC, NT], BF16, kind="ExternalOutput")

    with tile.TileContext(nc) as tc:
        with (
            tc.tile_pool(name="xpool", bufs=NN) as xpool,
            tc.tile_pool(name="hpool", bufs=HC + 4) as hpool,
            tc.tile_pool(name="w1hpool", bufs=HEAD) as w1hpool,
            tc.tile_pool(name="w1pool", bufs=4) as w1pool,
            tc.tile_pool(name="w2pool", bufs=2) as w2pool,
            tc.tile_pool(name="ybpool", bufs=NN) as ybpool,
            tc.tile_pool(name="ypool", bufs=4) as ypool,
            tc.tile_pool(name="cpool", bufs=E_LOC) as cpool,
            tc.tile_pool(name="wpool", bufs=1) as wpool,
            tc.tile_pool(name="ps1", bufs=4, space="PSUM") as ps1,
            tc.tile_pool(name="ps2", bufs=4, space="PSUM") as ps2,
        ):
            # HAM warm-up: throwaway matmuls on a zeroed scratch tile keep the
            # PE busy from preamble-exit (~7.2us) until the first chain's x/w1
            # DMAs land (~10.5us), so the clock gate reaches 8/8 (2.4 GHz)
            # before the real chains start.  These are in-order ahead of the
            # real chains (NX-paced ~107ns each cold), so the count must NOT
            # overshoot the DMA landing time.
            warm = wpool.tile([P, P], BF16)
            nc.gpsimd.memset(warm[:], 0.0)
            warm_acc = ps1.tile([P, NT], F32, name="acc", tag="acc")
            for _ in range(WARM_MM):
                nc.tensor.matmul(warm_acc[:, 0:P], warm[:], warm[:],
                                 start=True, stop=True)

            # Expert 0's first x token-half rides the sync ring right behind
            # the first w1 k-tile: HBM is the startup bottleneck, and the
            # first chain's deps must complete earliest.  Tile dependencies
            # are tracked per tile (a reader waits for ALL of a tile's
            # writers), so every startup-critical transfer gets its own tile:
            # per-k w1 head tiles and per-token-half x tiles.
            x_cur = [xpool.tile([P, DC, NT], BF16, name="x_t", tag="x_t")
                     for _ in range(NN)]

            for s in range(E_LOC):
                # ---------------- mm1: h = relu(x @ w1.T + b1) ----------------
                h_tiles = []
                w1h_tiles = []
                w1g_tiles = {}
                w2g_tiles = []
                n_head = HEAD if s == 0 else 0

                def load_w1_group(g, s=s):
                    t = w1pool.tile([P, W1G, DC, P], BF16, name="w1g", tag="w1g")
                    nc.sync.dma_start(out=t[:], in_=w1t[s, g])
                    w1g_tiles[g] = t

                def w1_slice(k, c, s=s, n_head=n_head):
                    if k < n_head:
                        return w1h_tiles[k][:, c, :]
                    return w1g_tiles[k // W1G][:, k % W1G, c, :]

                def mm1_chain(k, nt, s=s):
                    acc = ps1.tile([P, NT], F32, name="acc", tag="acc")
                    for c in range(DC):
                        nc.tensor.matmul(
                            acc[:],
                            w1_slice(k, c),
                            x_cur[nt][:, c, :],
                            start=(c == 0),
                            stop=(c == DC - 1),
                        )
                    nc.scalar.activation(
                        h_tiles[k][:, nt * NT : (nt + 1) * NT],
                        acc[:],
                        mybir.ActivationFunctionType.Relu,
                        bias=b_t[:, k : k + 1],
                    )

                if s == 0:
                    b_t = cpool.tile([P, HC + DC], F32)
                    nc.scalar.dma_start(out=b_t[:], in_=br[s])
                    # Sync-ring priority order: w1[0], then x-nt0 as ONE
                    # 8KB-row transfer (~390 GB/s; single_packet batches the
                    # descriptor stream so the 2KB-row head runs ~250 GB/s
                    # instead of ~175), w1[1..3] singles, the k4..7 group,
                    # x-nt1.  First chain is ready ~13us; secondary DMA
                    # rings are NOT used here - their cold first fetch lags
                    # unpredictably (1.5-3.5us) behind the descriptor post.
                    for k in range(HEAD):
                        t = w1hpool.tile([P, DC, P], BF16,
                                         name="w1h", tag="w1h")
                        nc.sync.dma_start(out=t[:], in_=w1h[k],
                                          single_packet=True)
                        w1h_tiles.append(t)
                        if k == 0:
                            nc.sync.dma_start(out=x_cur[0][:], in_=xt[0, 0],
                                              single_packet=True)
                    load_w1_group(1)
                    nc.sync.dma_start(out=x_cur[1][:], in_=xt[0, 1],
                                      single_packet=True)
                else:
                    b_t = cpool.tile([P, HC + DC], F32)
                    nc.scalar.dma_start(out=b_t[:], in_=br[s])
                    load_w1_group(0)
                    load_w1_group(1)

                for k in range(HC):
                    # Keep ~2 w1 groups of lookahead on the sync ring.
                    if k % W1G == 0:
                        g = k // W1G + 2
                        if g < NG1 and (g >= n_head // W1G):
                            load_w1_group(g)
                    # w2 group DMAs (2MB each) issue mid-mm1 so they hide
                    # under the trailing mm1 chains.
                    if k == 20 or k == 28:
                        t = w2pool.tile([P, W2G, HC, P], BF16,
                                        name="w2g", tag="w2g")
                        nc.sync.dma_start(out=t[:], in_=w2t[s, len(w2g_tiles)])
                        w2g_tiles.append(t)
                    h_tiles.append(hpool.tile([P, NE], BF16, name="h_t", tag="h_t"))
                    mm1_chain(k, 0)
                    if k >= DEFER:
                        mm1_chain(k - DEFER, 1)
                for k in range(HC - DEFER, HC):
                    mm1_chain(k, 1)

                # Prefetch next expert's x while this expert's mm2 runs
                # (xpool bufs=NN defers it until mm1(s) retires).
                if s + 1 < E_LOC:
                    x_next = [xpool.tile([P, DC, NT], BF16,
                                         name="x_t", tag="x_t")
                              for _ in range(NN)]
                    for nt in range(NN):
                        nc.scalar.dma_start(out=x_next[nt][:],
                                            in_=xt[s + 1, nt])
                    x_cur = x_next

                # ---------------- mm2: y = h @ w2.T + b2 ----------------
                # Evictions land in one bf16 y tile per token-half; each tile
                # is written back in a single 8KB-row DMA.  For the last
                # expert the final d-tile stays fine-grained (N=256 halves)
                # to keep the kernel tail short.
                y_big = [ybpool.tile([P, DC, NT], BF16, name="y_b", tag="y_b")
                         for _ in range(NN)]
                DD_BATCH = DC - 1 if s == E_LOC - 1 else DC

                for dd in range(DC):
                    if dd >= len(w2g_tiles) * W2G:
                        t = w2pool.tile([P, W2G, HC, P], BF16,
                                        name="w2g", tag="w2g")
                        nc.sync.dma_start(out=t[:], in_=w2t[s, len(w2g_tiles)])
                        w2g_tiles.append(t)
                    w2_t = w2g_tiles[dd // W2G]
                    v = dd % W2G
                    last = s == E_LOC - 1 and dd == DC - 1
                    acc2s = [ps2.tile([P, NT], F32, name="acc2", tag="acc2")
                             for _ in range(NN)]

                    if last:
                        # nt0: plain chain; its eviction + writeback hide under
                        # nt1's work.  nt1 runs as two N=256 column-half chains
                        # in separate PSUM banks: the first half's eviction and
                        # writeback hide under the second half's ~3.5us chain,
                        # and the post-final-matmul drainage (evict + DMA +
                        # completion receipt) covers only 256 columns.
                        for k in range(HC):
                            nc.tensor.matmul(
                                acc2s[0][:],
                                w2_t[:, v, k, :],
                                h_tiles[k][:, 0:NT],
                                start=(k == 0),
                                stop=(k == HC - 1),
                            )
                        y_tile = ypool.tile([P, NT], BF16)
                        nc.scalar.activation(
                            y_tile[:],
                            acc2s[0][:],
                            mybir.ActivationFunctionType.Identity,
                            bias=b_t[:, HC + dd : HC + dd + 1],
                        )
                        nc.scalar.dma_start(
                            out=yt[s, 0, :, dd, :],
                            in_=y_tile[:],
                            single_packet=True,
                        )
                        # Uneven split (384, 128): the first part's eviction
                        # and writeback hide under the second chain, and the
                        # post-final-matmul drainage covers only 128 columns.
                        HN0 = 384
                        accR = ps2.tile([P, NT], F32, name="acc2", tag="acc2")
                        for lo, hn, acch in ((0, HN0, acc2s[1]),
                                             (HN0, NT - HN0, accR)):
                            for k in range(HC):
                                nc.tensor.matmul(
                                    acch[:, 0:hn],
                                    w2_t[:, v, k, :],
                                    h_tiles[k][:, NT + lo : NT + lo + hn],
                                    start=(k == 0),
                                    stop=(k == HC - 1),
                                )
                            y_half = ypool.tile([P, NT], BF16)
                            nc.scalar.activation(
                                y_half[:, 0:hn],
                                acch[:, 0:hn],
                                mybir.ActivationFunctionType.Identity,
                                bias=b_t[:, HC + dd : HC + dd + 1],
                            )
                            nc.scalar.dma_start(
                                out=yt[s, 1, :, dd, lo : lo + hn],
                                in_=y_half[:, 0:hn],
                                single_packet=True,
                            )
                    else:
                        for k in range(HC):
                            for nt in range(NN):
                                nc.tensor.matmul(
                                    acc2s[nt][:],
                                    w2_t[:, v, k, :],
                                    h_tiles[k][:, nt * NT : (nt + 1) * NT],
                                    start=(k == 0),
                                    stop=(k == HC - 1),
                                )
                        for nt in range(NN):
                            nc.scalar.activation(
                                y_big[nt][:, dd, :],
                                acc2s[nt][:],
                                mybir.ActivationFunctionType.Identity,
                                bias=b_t[:, HC + dd : HC + dd + 1],
                            )
                        if dd == DD_BATCH - 1:
                            for nt in range(NN):
                                nc.scalar.dma_start(
                                    out=yt[s, nt, :, 0:DD_BATCH, :],
                                    in_=y_big[nt][:, 0:DD_BATCH, :],
                                    single_packet=True,
                                )

    nc.finalize()
    _dedupe_ldweights(nc)
    return nc


def _dedupe_ldweights(nc):
    """Drop PE InstLdweights that reload the weights already in the array.

    mm2 runs both token-half matmuls off the same w2 k-slice and the warmup
    matmuls reuse one zero tile, but every InstMatmult is split into its own
    LDWEIGHTS+MATMUL pair.  PE weights persist until overwritten, so a
    wait-free LDWEIGHTS whose source AP matches the previous one on the PE
    stream is a no-op - removing it shrinks the tensor instruction stream
    ~13%, and the runtime profiler injects one ~215ns timestamp slot per
    ~100 instructions, so fewer instructions directly cut those stalls.
    """
    for blk in nc.m.functions[0].blocks:
        ins = blk.instructions
        last_key = None
        to_del = []
        for i, x in enumerate(ins):
            tn = type(x).__name__
            if tn == "InstLdweights":
                key = str(x.concise())
                if key == last_key and not x.has_wait() and not x.has_update():
                    to_del.append(i)
                else:
                    last_key = key
            elif tn != "InstMatmult":
                # Only PE LDW/MM touch the weight array; any other PE
                # instruction is treated as clobbering (conservative).
                try:
                    if x.engine == mybir.EngineType.PE:
                        last_key = None
                except Exception:
                    last_key = None
        for i in reversed(to_del):
            del ins[i]


def _get_nc():
    if "nc" not in _CACHE:
        _CACHE["nc"] = _build_nc()
    return _CACHE["nc"]


def _prep_in_maps(xs, w1, b1, w2, b2):
    xs = np.asarray(xs, dtype=np.float32).astype(NP_BF16)
    w1 = np.asarray(w1, dtype=np.float32).astype(NP_BF16)
    b1 = np.asarray(b1, dtype=np.float32)
    w2 = np.asarray(w2, dtype=np.float32).astype(NP_BF16)
    b2 = np.asarray(b2, dtype=np.float32)

    x3 = xs.reshape(E, NE, D)
    in_maps = []
    for core in range(N_CORES):
        es = [E_LOC * core + s for s in range(E_LOC)]
        # xt[s, nt, p, c, j] = x_e[nt*512+j, c*128+p]
        xt = np.stack(
            [x3[e].reshape(NN, NT, DC, P).transpose(0, 3, 2, 1) for e in es]
        )
        # w1t[s, g, p, u, c, j] = w1[e, (g*4+u)*128+j, c*128+p]
        w1t = np.stack(
            [w1[e].reshape(NG1, W1G, P, DC, P).transpose(0, 4, 1, 3, 2)
             for e in es]
        )
        # w1h[k, p, c, j] = w1[e0, k*128+j, c*128+p] for k < HEAD
        w1hm = (w1[es[0]][: HEAD * P].reshape(HEAD, P, DC, P)
                .transpose(0, 3, 2, 1))
        # w2t[s, g, p, v, k, j] = w2[e, (g*2+v)*128+j, k*128+p]
        w2t = np.stack(
            [w2[e].reshape(NG2, W2G, P, HC, P).transpose(0, 4, 1, 3, 2)
             for e in es]
        )
        # br[s, p, k] = b1[e, k*128+p];  br[s, p, HC+dd] = b2[e, dd*128+p]
        brm = np.stack(
            [np.concatenate(
                [b1[e].reshape(HC, P).T, b2[e].reshape(DC, P).T], axis=1)
             for e in es]
        )
        in_maps.append(
            {
                "xt": np.ascontiguousarray(xt),
                "w1h": np.ascontiguousarray(w1hm),
                "w1t": np.ascontiguousarray(w1t),
                "w2t": np.ascontiguousarray(w2t),
                "br": np.ascontiguousarray(brm),
            }
        )
    return in_maps


def _gather(results):
    y = np.empty((N_TOK, D), dtype=np.float32)
    for core in range(N_CORES):
        out = results[core]["yt"]  # [E_LOC, NN, P, DC, NT] bf16
        for s in range(E_LOC):
            e = E_LOC * core + s
            # yt[s, nt, p, dd, j] = y_e[nt*512+j, dd*128+p]
            for nt in range(NN):
                y[e * NE + nt * NT : e * NE + (nt + 1) * NT] = (
                    out[s, nt].transpose(2, 1, 0).reshape(NT, D)
                    .astype(np.float32)
                )
    return y


def _run(in_maps, **kwargs):
    nc = _get_nc()
    return run_bass_kernel_spmd(nc, in_maps, core_ids=list(range(N_CORES)), **kwargs)


def kernel(xs, fwd_expert_count, w1, b1, w2, b2):
    # fwd_expert_count is uniform (N_TOK // E per expert) by construction,
    # matching the reference, which also hardcodes the uniform grouping.
    in_maps = _prep_in_maps(xs, w1, b1, w2, b2)
    res = _run(in_maps)
    return _gather(res.results)
